# revision 15
# baseline (speedup 1.0000x reference)
"""Trainium2 Bass kernel for nn_AttentionSeqModel (B=1024,L=256,D=256,H=128,A=2).

Data-parallel over batch: 128 rows per core on 8 NeuronCores, no collectives.
Layout convention on-chip: activations transposed [feature(part), batch(free)].
"""
import functools
import os

import numpy as np
import ml_dtypes

import concourse.bass as bass
import concourse.bacc as bacc_mod
import concourse.tile as tile
from concourse import mybir
from concourse.bass import ds, ts
from concourse.bass_utils import run_bass_kernel_spmd

BF16 = ml_dtypes.bfloat16
B, L, D, H, A = 1024, 256, 256, 128, 2
NCORES = 8
BS = B // NCORES  # 128 batch rows per core
NSWAP = 6
dt = mybir.dt
AF = mybir.ActivationFunctionType
ALU = mybir.AluOpType


# ---------------------------------------------------------------- graph build
def build_graph(stage=3):
    nc = bacc_mod.Bacc()
    bf, f32 = dt.bfloat16, dt.float32

    def P(name, shape, dtype, isOutput=False):
        return nc.declare_dram_parameter(name, shape, dtype, isOutput)

    obsT = P("obsT", [L, 2, 128, BS], bf)       # [l, dhalf, d128, b]
    v1ch = P("v1ch", [L, 2, 128, 128], bf)      # v1_W.T chunks [l, j, k, m]
    eembT = P("eembT", [2, 128, H], bf)         # enc_emb_W.T halves [j, d, h]
    eembB = P("eembB", [H, 1], f32)
    v1b = P("v1b", [128, 1], f32)
    v2T = P("v2T", [128, 128], bf)
    v2b = P("v2b", [128, 1], f32)
    v3c = P("v3c", [128, 1], bf)
    v3b = P("v3b", [1, 1], f32)
    wihT = P("wihT", [3, H, H], bf)             # enc gate weights (r,z,n), [g, k, m]
    whhT = P("whhT", [3, H, H], bf)
    ebr = P("ebr", [H, 1], f32)
    ebz = P("ebz", [H, 1], f32)
    ebin = P("ebin", [H, 1], f32)
    ebhn = P("ebhn", [H, 1], f32)
    am1c = P("am1c", [3, L], bf)                # folded dec_emb->attn + bias row
    whT = P("whT", [H, L], bf)                  # attn_W[:,H:].T
    maskTd = P("maskT", [2, 128, L], f32)        # mask.T split [j, l128, t]
    cm1c = P("cm1c", [3, H], bf)                # folded dec_emb->comb + bias row
    caT = P("caT", [H, H], bf)
    dihT = P("dihT", [3, H, H], bf)
    dhhT = P("dhhT", [3, H, H], bf)
    dbr = P("dbr", [H, 1], f32)
    dbz = P("dbz", [H, 1], f32)
    dbin = P("dbin", [H, 1], f32)
    dbhn = P("dbhn", [H, 1], f32)
    owT = P("owT", [H, A], bf)
    ob = P("ob", [A, 1], f32)
    ident = P("ident", [128, 128], bf)
    onesr = P("onesr", [1, BS], bf)
    out_y = P("out_y", [L, A, BS], bf, isOutput=True)
    out_v = P("out_v", [1, BS], f32, isOutput=True)

    encD = nc.dram_tensor("encD", [L, BS, H], bf)  # encoder outputs [l, b, h]
    embD = nc.dram_tensor("embD", [L, H, BS], bf)  # embeddings [l, h, b]

    with tile.TileContext(nc) as tc:
        with (
            tc.tile_pool(name="const", bufs=1) as const,
            tc.tile_pool(name="state", bufs=1) as state,
            tc.tile_pool(name="p1", bufs=4) as p1,
            tc.tile_pool(name="emx", bufs=3) as emx,
            tc.tile_pool(name="wk", bufs=3) as wk,
            tc.tile_pool(name="ps", bufs=7, space="PSUM") as psp,
            tc.tile_pool(name="pv", bufs=1, space="PSUM") as pvp,
        ):
            # ---- load constants into SBUF
            def cload(name, dram, shape, dtype=bf, rearr=None):
                t = const.tile(shape, dtype, tag=name)
                src = dram[:]
                if rearr is not None:
                    src = src.rearrange(rearr)
                nc.sync.dma_start(out=t, in_=src)
                return t

            eembT_t = cload("eembT", eembT, [128, 2, H], rearr="j d h -> d j h")
            eembB_t = cload("eembB", eembB, [H, 1], f32)
            v1b_t = cload("v1b", v1b, [128, 1], f32)
            v2T_t = cload("v2T", v2T, [128, 128])
            v2b_t = cload("v2b", v2b, [128, 1], f32)
            v3c_t = cload("v3c", v3c, [128, 1])
            v3b_t = cload("v3b", v3b, [1, 1], f32)
            wih_t = cload("wih", wihT, [128, 3, H], rearr="g k m -> k g m")
            whh_t = cload("whh", whhT, [128, 3, H], rearr="g k m -> k g m")
            ebr_t = cload("ebr", ebr, [H, 1], f32)
            ebz_t = cload("ebz", ebz, [H, 1], f32)
            ebin_t = cload("ebin", ebin, [H, 1], f32)
            ebhn_t = cload("ebhn", ebhn, [H, 1], f32)
            am1c_t = cload("am1c", am1c, [3, L])
            whT_t = cload("whT", whT, [H, L])
            mask_t = cload("maskT", maskTd, [128, 2, L], dt.float32, rearr="j l t -> l j t")
            cm1c_t = cload("cm1c", cm1c, [3, H])
            caT_t = cload("caT", caT, [H, H])
            dih_t = cload("dih", dihT, [128, 3, H], rearr="g k m -> k g m")
            dhh_t = cload("dhh", dhhT, [128, 3, H], rearr="g k m -> k g m")
            dbr_t = cload("dbr", dbr, [H, 1], f32)
            dbz_t = cload("dbz", dbz, [H, 1], f32)
            dbin_t = cload("dbin", dbin, [H, 1], f32)
            dbhn_t = cload("dbhn", dbhn, [H, 1], f32)
            owT_t = cload("owT", owT, [H, A])
            ob_t = cload("ob", ob, [A, 1], f32)
            ident_t = cload("ident", ident, [128, 128])

            # ---- persistent state
            hT = state.tile([H, BS], bf, tag="hT")
            inp_aug = state.tile([3, BS], bf, tag="inp_aug")
            encA0 = state.tile([128, BS, H], bf, tag="encA0")    # [l(0:128), b, h]
            encA1 = state.tile([128, BS, H], bf, tag="encA1")

            # ================= pass 1: value branch + embeddings =============
            vps = pvp.tile([128, BS], dt.float32, tag="vps")
            EMB_CHUNK = 4
            for l0 in range(0, L, EMB_CHUNK):
                embp = psp.tile([H, EMB_CHUNK * BS], dt.float32, tag="ps")
                obs_t = p1.tile([128, EMB_CHUNK, 2, BS], bf, tag="obs")
                nc.gpsimd.dma_start(
                    out=obs_t,
                    in_=obsT[l0:l0 + EMB_CHUNK].rearrange("l j d b -> d l j b"),
                )
                v1c_t = p1.tile([128, EMB_CHUNK, 2, 128], bf, tag="v1c")
                nc.gpsimd.dma_start(
                    out=v1c_t,
                    in_=v1ch[l0:l0 + EMB_CHUNK].rearrange("l j k m -> k l j m"),
                )
                for li in range(EMB_CHUNK):
                    l = l0 + li
                    for j in range(2):
                        nc.tensor.matmul(
                            vps, v1c_t[:, li, j, :], obs_t[:, li, j, :],
                            start=(l == 0 and j == 0), stop=(l == L - 1 and j == 1),
                        )
                        nc.tensor.matmul(
                            embp[:, li * BS:(li + 1) * BS],
                            eembT_t[:, j, :], obs_t[:, li, j, :],
                            start=(j == 0), stop=(j == 1),
                        )
                # evict emb chunk (+bias) to DRAM
                emb_sb = p1.tile([H, EMB_CHUNK, BS], bf, tag="embs")
                nc.vector.tensor_scalar(
                    out=emb_sb, in0=embp,
                    scalar1=eembB_t, scalar2=None, op0=ALU.add,
                )
                nc.gpsimd.dma_start(
                    out=embD[l0:l0 + EMB_CHUNK].rearrange("l h b -> h l b"),
                    in_=emb_sb,
                )
            # value head
            v1_sb = wk.tile([128, BS], bf, tag="v1s")
            nc.scalar.activation(v1_sb, vps, AF.Relu, bias=v1b_t)
            v2ps = psp.tile([128, BS], dt.float32, tag="ps")
            nc.tensor.matmul(v2ps, v2T_t, v1_sb, start=True, stop=True)
            v2_sb = wk.tile([128, BS], bf, tag="v2s")
            nc.scalar.activation(v2_sb, v2ps, AF.Relu, bias=v2b_t)
            v3ps = psp.tile([1, BS], dt.float32, tag="ps")
            nc.tensor.matmul(v3ps, v3c_t, v2_sb, start=True, stop=True)
            v_sb = wk.tile([1, BS], dt.float32, tag="vs")
            nc.scalar.activation(v_sb, v3ps, AF.Identity, bias=v3b_t)
            nc.gpsimd.dma_start(out=out_v[:], in_=v_sb)

            # ================= encoder GRU ===================================
            nc.vector.memset(hT, 0.0)

            def enc_step(x, hbt_dst):
                """one encoder GRU step; x=[H,BS] input, updates hT,
                writes transposed h into hbt_dst [BS,H]."""
                rp = psp.tile([H, BS], dt.float32, tag="ps")
                nc.tensor.matmul(rp, wih_t[:, 0, :], x, start=True, stop=False)
                nc.tensor.matmul(rp, whh_t[:, 0, :], hT, start=False, stop=True)
                zp = psp.tile([H, BS], dt.float32, tag="ps")
                nc.tensor.matmul(zp, wih_t[:, 1, :], x, start=True, stop=False)
                nc.tensor.matmul(zp, whh_t[:, 1, :], hT, start=False, stop=True)
                inp_ = psp.tile([H, BS], dt.float32, tag="ps")
                nc.tensor.matmul(inp_, wih_t[:, 2, :], x, start=True, stop=True)
                hnp = psp.tile([H, BS], dt.float32, tag="ps")
                nc.tensor.matmul(hnp, whh_t[:, 2, :], hT, start=True, stop=True)
                r_sb = wk.tile([H, BS], bf, tag="r")
                nc.scalar.activation(r_sb, rp, AF.Sigmoid, bias=ebr_t)
                z_sb = wk.tile([H, BS], bf, tag="z")
                nc.scalar.activation(z_sb, zp, AF.Sigmoid, bias=ebz_t)
                hnb = wk.tile([H, BS], bf, tag="hnb")
                nc.vector.tensor_scalar(
                    out=hnb, in0=hnp, scalar1=ebhn_t, scalar2=None, op0=ALU.add)
                rhn = wk.tile([H, BS], bf, tag="rhn")
                nc.vector.tensor_tensor(out=rhn, in0=r_sb, in1=hnb, op=ALU.mult)
                npre = wk.tile([H, BS], bf, tag="npre")
                nc.vector.tensor_tensor(out=npre, in0=rhn, in1=inp_, op=ALU.add)
                n_sb = wk.tile([H, BS], bf, tag="n")
                nc.scalar.activation(n_sb, npre, AF.Tanh, bias=ebin_t)
                d_sb = wk.tile([H, BS], bf, tag="d")
                nc.vector.tensor_tensor(out=d_sb, in0=hT, in1=n_sb, op=ALU.subtract)
                m_sb = wk.tile([H, BS], bf, tag="m")
                nc.vector.tensor_tensor(out=m_sb, in0=z_sb, in1=d_sb, op=ALU.mult)
                nc.vector.tensor_tensor(out=hT, in0=n_sb, in1=m_sb, op=ALU.add)
                # transpose h -> [b, h], stash to DRAM for the attention layout
                hbt_ps = psp.tile([BS, H], bf, tag="ps")
                nc.tensor.transpose(hbt_ps, hT, ident_t)
                nc.scalar.activation(hbt_dst, hbt_ps, AF.Copy)

            # peel steps 0..7 (0..5 read swapped rows)
            if stage < 1:
                skip = True
            else:
                skip = False
            embx0 = emx.tile([H, 8, BS], bf, tag="embx")
            for t in range(8 if not skip else 0):
                row = (L - 1 - t) if t < NSWAP else t
                nc.gpsimd.dma_start(out=embx0[:, t, :], in_=embD[row])
            hbt0 = emx.tile([BS, 8, H], bf, tag="hbt8")
            for t in range(8 if not skip else 0):
                enc_step(embx0[:, t, :], hbt0[:, t, :])
            if not skip:
                nc.gpsimd.dma_start(
                    out=encD[0:8].rearrange("l b h -> b l h"), in_=hbt0)
            if stage >= 2:
                with tc.For_i(8, L, 8) as it:
                    embx8 = emx.tile([H, 8, BS], bf, tag="embx")
                    nc.gpsimd.dma_start(
                        out=embx8,
                        in_=embD[ds(it, 8)].rearrange("l h b -> h l b"))
                    hbt8 = emx.tile([BS, 8, H], bf, tag="hbt8")
                    for k in range(8):
                        enc_step(embx8[:, k, :], hbt8[:, k, :])
                    nc.gpsimd.dma_start(
                        out=encD[ds(it, 8)].rearrange("l b h -> b l h"),
                        in_=hbt8)

            # load attention value layout [l, b, h]
            nc.gpsimd.dma_start(out=encA0, in_=encD[0:128])
            nc.gpsimd.dma_start(out=encA1, in_=encD[128:256])

            # ================= decoder ======================================
            nc.vector.memset(inp_aug[0:2, :], 0.0)
            nc.gpsimd.dma_start(out=inp_aug[2:3, :], in_=onesr[:])

            def dec_step(tv, y_dst):
                # logits [b, l] = inp_aug.T @ am1c + h.T @ whT
                lg = psp.tile([BS, L], dt.float32, tag="ps")
                nc.tensor.matmul(lg, inp_aug, am1c_t, start=True, stop=False)
                nc.tensor.matmul(lg, hT, whT_t, start=False, stop=True)
                p_sb = wk.tile([BS, L], bf, tag="p")
                zs = wk.tile([BS, 1], dt.float32, tag="zs")
                nc.scalar.activation(p_sb, lg, AF.Exp, accum_out=zs)
                zr = wk.tile([BS, 1], dt.float32, tag="zr")
                nc.vector.reciprocal(zr, zs)
                pn = wk.tile([BS, L], bf, tag="pn")
                nc.vector.tensor_scalar(
                    out=pn, in0=p_sb, scalar1=zr, scalar2=None, op0=ALU.mult)
                # transpose softmax weights, apply mask column tv
                wT0 = wk.tile([128, BS], bf, tag="wT0")
                wT1 = wk.tile([128, BS], bf, tag="wT1")
                for j, wTj in ((0, wT0), (1, wT1)):
                    ptp = psp.tile([128, BS], bf, tag="ps")
                    nc.tensor.transpose(ptp, pn[:, j * 128:(j + 1) * 128], ident_t)
                    nc.vector.tensor_scalar(
                        out=wTj, in0=ptp, scalar1=mask_t[:, j, ds(tv, 1)],
                        scalar2=None, op0=ALU.mult)
                # attention apply: per-b matvec, PSUM accumulate over l halves
                app = psp.tile([H, BS], dt.float32, tag="ps")
                for b in range(BS):
                    nc.tensor.matmul(
                        app[:, b:b + 1], encA0[:, b, :], wT0[:, b:b + 1],
                        start=True, stop=False)
                    nc.tensor.matmul(
                        app[:, b:b + 1], encA1[:, b, :], wT1[:, b:b + 1],
                        start=False, stop=True)
                ap_sb = wk.tile([H, BS], bf, tag="ap")
                nc.scalar.activation(ap_sb, app, AF.Copy)
                # g = tanh(comb)
                gp = psp.tile([H, BS], dt.float32, tag="ps")
                nc.tensor.matmul(gp, cm1c_t, inp_aug, start=True, stop=False)
                nc.tensor.matmul(gp, caT_t, ap_sb, start=False, stop=True)
                g_sb = wk.tile([H, BS], bf, tag="g")
                nc.scalar.activation(g_sb, gp, AF.Tanh)
                # GRU cell (x=g_sb, h=hT)
                rp = psp.tile([H, BS], dt.float32, tag="ps")
                nc.tensor.matmul(rp, dih_t[:, 0, :], g_sb, start=True, stop=False)
                nc.tensor.matmul(rp, dhh_t[:, 0, :], hT, start=False, stop=True)
                zp = psp.tile([H, BS], dt.float32, tag="ps")
                nc.tensor.matmul(zp, dih_t[:, 1, :], g_sb, start=True, stop=False)
                nc.tensor.matmul(zp, dhh_t[:, 1, :], hT, start=False, stop=True)
                inp_ = psp.tile([H, BS], dt.float32, tag="ps")
                nc.tensor.matmul(inp_, dih_t[:, 2, :], g_sb, start=True, stop=True)
                hnp = psp.tile([H, BS], dt.float32, tag="ps")
                nc.tensor.matmul(hnp, dhh_t[:, 2, :], hT, start=True, stop=True)
                r_sb = wk.tile([H, BS], bf, tag="r")
                nc.scalar.activation(r_sb, rp, AF.Sigmoid, bias=dbr_t)
                z_sb = wk.tile([H, BS], bf, tag="z")
                nc.scalar.activation(z_sb, zp, AF.Sigmoid, bias=dbz_t)
                hnb = wk.tile([H, BS], bf, tag="hnb")
                nc.vector.tensor_scalar(
                    out=hnb, in0=hnp, scalar1=dbhn_t, scalar2=None, op0=ALU.add)
                rhn = wk.tile([H, BS], bf, tag="rhn")
                nc.vector.tensor_tensor(out=rhn, in0=r_sb, in1=hnb, op=ALU.mult)
                npre = wk.tile([H, BS], bf, tag="npre")
                nc.vector.tensor_tensor(out=npre, in0=rhn, in1=inp_, op=ALU.add)
                n_sb = wk.tile([H, BS], bf, tag="n")
                nc.scalar.activation(n_sb, npre, AF.Tanh, bias=dbin_t)
                d_sb = wk.tile([H, BS], bf, tag="d")
                nc.vector.tensor_tensor(out=d_sb, in0=hT, in1=n_sb, op=ALU.subtract)
                m_sb = wk.tile([H, BS], bf, tag="m")
                nc.vector.tensor_tensor(out=m_sb, in0=z_sb, in1=d_sb, op=ALU.mult)
                nc.vector.tensor_tensor(out=hT, in0=n_sb, in1=m_sb, op=ALU.add)
                # y = tanh(out_W @ h + b); write to staging and feed back
                yp = psp.tile([A, BS], dt.float32, tag="ps")
                nc.tensor.matmul(yp, owT_t, hT, start=True, stop=True)
                nc.scalar.activation(y_dst, yp, AF.Tanh, bias=ob_t)
                nc.vector.tensor_copy(out=inp_aug[0:2, :], in_=y_dst)

            if stage >= 3:
              with tc.For_i(0, L, 2, hint_engines=(mybir.EngineType.PE,)) as it:
                yT2 = wk.tile([A, 2 * BS], bf, tag="yT2")
                dec_step(it, yT2[:, 0:BS])
                dec_step(it + 1, yT2[:, BS:2 * BS])
                nc.gpsimd.dma_start(
                    out=out_y[ds(it, 2)].rearrange("t a b -> a t b"),
                    in_=yT2.rearrange("a (t b) -> a t b", t=2),
                )
    nc.finalize()
    return nc


@functools.lru_cache(maxsize=1)
def _graph():
    return build_graph()


# ---------------------------------------------------------------- host side
def _prep_shared(i):
    """fold/transpose weights (shared across cores)."""
    bf = lambda x: np.ascontiguousarray(x, dtype=BF16)
    f32 = lambda x: np.ascontiguousarray(x, dtype=np.float32)
    enc_Wih, enc_Whh = i["enc_Wih"], i["enc_Whh"]
    enc_bih, enc_bhh = i["enc_bih"], i["enc_bhh"]
    dec_Wih, dec_Whh = i["dec_Wih"], i["dec_Whh"]
    dec_bih, dec_bhh = i["dec_bih"], i["dec_bhh"]
    attn_W, attn_b = i["attn_W"], i["attn_b"]
    comb_W, comb_b = i["comb_W"], i["comb_b"]
    dec_emb_W, dec_emb_b = i["dec_emb_W"], i["dec_emb_b"]
    We, Wh = attn_W[:, :H], attn_W[:, H:]
    Ce, Ca = comb_W[:, :H], comb_W[:, H:]
    AM1 = (We @ dec_emb_W).T                       # [A, L]
    c1 = dec_emb_b @ We.T + attn_b                 # [L]
    CM1 = (Ce @ dec_emb_W).T                       # [A, H]
    c2 = dec_emb_b @ Ce.T + comb_b                 # [H]
    gstk = lambda W: np.stack([W[g * H:(g + 1) * H].T for g in range(3)])
    m = {
        "v1ch": bf(i["v1_W"].T.reshape(L, 2, 128, 128)),
        "eembT": bf(i["enc_emb_W"].T.reshape(2, 128, H)),
        "eembB": f32(i["enc_emb_b"][:, None]),
        "v1b": f32(i["v1_b"][:, None]),
        "v2T": bf(i["v2_W"].T),
        "v2b": f32(i["v2_b"][:, None]),
        "v3c": bf(i["v3_W"].T),
        "v3b": f32(i["v3_b"][:, None]),
        "wihT": bf(gstk(enc_Wih)),
        "whhT": bf(gstk(enc_Whh)),
        "ebr": f32((enc_bih[0:H] + enc_bhh[0:H])[:, None]),
        "ebz": f32((enc_bih[H:2 * H] + enc_bhh[H:2 * H])[:, None]),
        "ebin": f32(enc_bih[2 * H:][:, None]),
        "ebhn": f32(enc_bhh[2 * H:][:, None]),
        "am1c": bf(np.stack([AM1[0], AM1[1], c1])),
        "whT": bf(Wh.T),
        "maskT": f32(np.ascontiguousarray(i["attn_mask"].T).reshape(2, 128, L)),
        "cm1c": bf(np.stack([CM1[0], CM1[1], c2])),
        "caT": bf(Ca.T),
        "dihT": bf(gstk(dec_Wih)),
        "dhhT": bf(gstk(dec_Whh)),
        "dbr": f32((dec_bih[0:H] + dec_bhh[0:H])[:, None]),
        "dbz": f32((dec_bih[H:2 * H] + dec_bhh[H:2 * H])[:, None]),
        "dbin": f32(dec_bih[2 * H:][:, None]),
        "dbhn": f32(dec_bhh[2 * H:][:, None]),
        "owT": bf(i["out_W"].T),
        "ob": f32(i["out_b"][:, None]),
        "ident": bf(np.eye(128)),
        "onesr": bf(np.ones((1, BS))),
    }
    return m


def _run(inputs, trace=False):
    shared = _prep_shared(inputs)
    obs = np.asarray(inputs["obs"], np.float32)
    in_maps = []
    for c in range(NCORES):
        m = dict(shared)
        oc = obs[c * BS:(c + 1) * BS]                       # [BS, L, D]
        m["obsT"] = np.ascontiguousarray(
            oc.transpose(1, 2, 0), dtype=BF16).reshape(L, 2, 128, BS)
        in_maps.append(m)
    nc = _graph()
    return run_bass_kernel_spmd(nc, in_maps, list(range(NCORES)), trace=trace)


def _postprocess(results):
    outs = np.empty((B, L, A), np.float32)
    val = np.empty((B, 1), np.float32)
    for c in range(NCORES):
        y = np.asarray(results[c]["out_y"]).astype(np.float32)  # [L, A, BS]
        outs[c * BS:(c + 1) * BS] = y.transpose(2, 0, 1)
        val[c * BS:(c + 1) * BS, 0] = np.asarray(
            results[c]["out_v"], np.float32)[0]
    outs[:, :NSWAP] = outs[:, L - NSWAP:][:, ::-1]
    return outs.reshape(B, L * A), val


def kernel(**inputs):
    res = _run(inputs, trace=False)
    return _postprocess(res.results)
